# revision 29
# baseline (speedup 1.0000x reference)
"""MoE FFN (8 experts, top-2) Trainium2 Bass kernel.

Strategy: token-sharded data parallel over 8 cores. Core i handles tokens
[i*1024, (i+1)*1024) of the flattened [B*S=8192, D=1024] input. The tiny
router (0.06% of FLOPs) runs on host in exact fp32 (matching the reference's
op order, so top-2 selection is bit-stable); the dense FFN compute runs on
device: for each of the 8 experts, hT = gelu_tanh(w1 @ x + b1) over all local
tokens, then out += gate_e * (w2.T @ hT + b2), accumulated in SBUF.
All FFN matmuls use float32r (fp32 bits, 4x-faster PE mode, full rate at
moving dim >= 256).

Everything lives in [feature, token] layout on chip so no transposes are
needed: fc1 consumes xT tiles as the moving operand and w1T tiles as
stationary; fc2 consumes hT tiles as stationary and w2T tiles as moving,
producing [token, d] PSUM tiles so the per-token gate is a per-partition
scalar multiply fused with the cross-expert accumulation (one DVE op).
"""

import numpy as np
from contextlib import ExitStack

import concourse.bass as bass
import concourse.bacc as bacc
import concourse.tile as tile
from concourse import mybir
from concourse.bass_utils import run_bass_kernel_spmd

FR = mybir.dt.float32r
F32 = mybir.dt.float32
AF = mybir.ActivationFunctionType
OP = mybir.AluOpType

NCORES = 8
E = 8            # experts
D = 1024         # model dim
H = 4096         # hidden dim
TLOC = 1024      # tokens per core
CHUNK = 512      # tokens per hT block
NCH = TLOC // CHUNK
TT = CHUNK // 128        # token tiles per chunk (4)
DS = D // 128            # d sub-blocks (8)
NHT = H // 128           # h tiles (32)
W1G = H // 512           # 8 w1 DMA groups per expert, each [128, DS, 512]
DC = D // 512            # 2 output d chunks
HQ = 4                   # w2 h-quarters, each 8 h-tiles


def build_nc():
    nc = bacc.Bacc("TRN2", target_bir_lowering=False, debug=False,
                   num_devices=NCORES)
    xh = nc.dram_tensor("xh", [128, DS, TLOC], FR, kind="ExternalInput")
    w1h = nc.dram_tensor("w1h", [E, W1G, 128, DS, 512], FR, kind="ExternalInput")
    w2h = nc.dram_tensor("w2h", [E, DC, HQ, 128, 8, 512], FR, kind="ExternalInput")
    b1h = nc.dram_tensor("b1h", [128, E, NHT], F32, kind="ExternalInput")
    b2h = nc.dram_tensor("b2h", [E, D], FR, kind="ExternalInput")
    # host-computed gates: gh[p, tt_global, e] (token t = tt_global*128 + p)
    gh = nc.dram_tensor("gh", [128, TLOC // 128, E], F32, kind="ExternalInput")
    # transposed gates for the fc2-bias rank-1 term: ght[e, tok]
    ght = nc.dram_tensor("ght", [E, TLOC], FR, kind="ExternalInput")
    outd = nc.dram_tensor("outd", [NCH, 128, TT, DC, 512], F32,
                          kind="ExternalOutput")

    with tile.TileContext(nc) as tc, ExitStack() as ctx:
        const = ctx.enter_context(tc.tile_pool(name="const", bufs=1))
        hpool = ctx.enter_context(tc.tile_pool(name="hT", bufs=1))
        apool = ctx.enter_context(tc.tile_pool(name="oacc", bufs=2))
        w1p = ctx.enter_context(tc.tile_pool(name="w1", bufs=2))
        w2p = ctx.enter_context(tc.tile_pool(name="w2", bufs=2))
        ps1 = ctx.enter_context(tc.tile_pool(name="ps1", bufs=2, space="PSUM"))
        ps2 = ctx.enter_context(tc.tile_pool(name="ps2", bufs=6, space="PSUM"))

        # --- resident tensors ---
        xsb = const.tile([128, DS, TLOC], FR)
        nc.sync.dma_start(out=xsb[:], in_=xh[:, :, :])
        b1sb = const.tile([128, E, NHT], F32)
        nc.sync.dma_start(out=b1sb[:], in_=b1h[:, :, :])
        b2sb = const.tile([E, D], FR)
        nc.sync.dma_start(out=b2sb[:], in_=b2h[:, :])
        gsb = const.tile([128, TLOC // 128, E], F32)
        nc.sync.dma_start(out=gsb[:], in_=gh[:, :, :])
        gtsb = const.tile([E, TLOC], FR)
        nc.sync.dma_start(out=gtsb[:], in_=ght[:, :])

        for c in range(NCH):
            t0 = c * CHUNK
            # init oacc with the fc2 bias term: oacc[t, d] = sum_e g_e(t) b2_e(d)
            oacc = apool.tile([128, TT, DC, 512], F32)
            for tt in range(TT):
                for dc in range(DC):
                    pb = ps2.tile([128, 512], F32, name=f"pb{tt}_{dc}", tag="pst")
                    nc.tensor.matmul(
                        pb[:],
                        lhsT=gtsb[:, t0 + tt * 128: t0 + (tt + 1) * 128],
                        rhs=b2sb[:, dc * 512: (dc + 1) * 512],
                        start=True, stop=True,
                    )
                    nc.vector.tensor_copy(oacc[:, tt, dc, :], pb[:])

            for e in range(E):
                # ---------------- fc1: hT[h, tok] = gelu(w1 @ x + b1) --------
                hT = hpool.tile([128, NHT, CHUNK], FR)
                for wg in range(W1G):  # 8 groups x 4 h-tiles
                    w1t = w1p.tile([128, DS, 512], FR)
                    nc.sync.dma_start(out=w1t[:], in_=w1h[e, wg, :, :, :])
                    for hti in range(4):
                        ht = wg * 4 + hti
                        p1 = ps1.tile([128, 512], F32)
                        for ds in range(DS):
                            nc.tensor.matmul(
                                p1[:, :CHUNK],
                                lhsT=w1t[:, ds, hti * 128: (hti + 1) * 128],
                                rhs=xsb[:, ds, t0: t0 + CHUNK],
                                start=(ds == 0),
                                stop=(ds == DS - 1),
                            )
                        nc.scalar.activation(
                            hT[:, ht, :], p1[:, :CHUNK], AF.Gelu_apprx_tanh,
                            bias=b1sb[:, e, ht: ht + 1],
                        )
                # ---------------- fc2: out[tok, d] += g_e * (hT.T @ w2) ------
                for dc in range(DC):
                    pst = [ps2.tile([128, 512], F32, name=f"pst{_t}", tag="pst")
                           for _t in range(TT)]
                    for hq in range(HQ):
                        w2t = w2p.tile([128, 8, 512], FR)
                        nc.sync.dma_start(out=w2t[:], in_=w2h[e, dc, hq, :, :, :])
                        for hh in range(8):
                            ht = hq * 8 + hh
                            for tt in range(TT):
                                nc.tensor.matmul(
                                    pst[tt][:],
                                    lhsT=hT[:, ht, tt * 128: (tt + 1) * 128],
                                    rhs=w2t[:, hh, :],
                                    start=(hq == 0 and hh == 0),
                                    stop=(hq == HQ - 1 and hh == 7),
                                )
                    for tt in range(TT):
                        nc.vector.scalar_tensor_tensor(
                            out=oacc[:, tt, dc, :],
                            in0=pst[tt][:],
                            scalar=gsb[:, (t0 // 128) + tt, e: e + 1],
                            in1=oacc[:, tt, dc, :],
                            op0=OP.mult,
                            op1=OP.add,
                        )
            nc.sync.dma_start(out=outd[c, :, :, :, :], in_=oacc[:])
    nc.compile()
    return nc


CAP = 384                # routed capacity per (core, expert): 3 token tiles
NT = CAP // 128
TLOC1 = TLOC + 1         # +1 dummy row for padded scatter slots


def build_nc_routed():
    """Routed variant: each expert computes only its own tokens.

    Host supplies per-expert gather indices (into the core's local x rows),
    scatter indices (row in the padded output; CAP-padding slots point at the
    dummy row TLOC), and gathered gates. Device: indirect-DMA gather -> PE
    transpose -> fc1 -> fc2 (+bias via K=1 ones matmul) -> gate-scale ->
    indirect scatter-ADD straight into the (pre-zeroed) padded output.
    """
    nc = bacc.Bacc("TRN2", target_bir_lowering=False, debug=False,
                   num_devices=NCORES)
    xrowd = nc.dram_tensor("xrowd", [TLOC, D], FR, kind="ExternalInput")
    w1h = nc.dram_tensor("w1h", [E, W1G, 128, DS, 512], FR, kind="ExternalInput")
    w2h = nc.dram_tensor("w2h", [E, DC, HQ, 128, 8, 512], FR, kind="ExternalInput")
    b1h = nc.dram_tensor("b1h", [128, E, NHT], F32, kind="ExternalInput")
    b2f = nc.dram_tensor("b2f", [1, E * D], FR, kind="ExternalInput")
    onesd = nc.dram_tensor("onesd", [1, 128], FR, kind="ExternalInput")
    idxh = nc.dram_tensor("idxh", [128, E, NT], mybir.dt.int32,
                          kind="ExternalInput")
    sidxh = nc.dram_tensor("sidxh", [128, E, NT], mybir.dt.int32,
                           kind="ExternalInput")
    g2h = nc.dram_tensor("g2h", [128, E, NT], F32, kind="ExternalInput")
    identd = nc.dram_tensor("identd", [128, 128], FR, kind="ExternalInput")
    outd = nc.dram_tensor("outd", [TLOC1, D], F32, kind="ExternalOutput")

    with tile.TileContext(nc) as tc, ExitStack() as ctx:
        const = ctx.enter_context(tc.tile_pool(name="const", bufs=1))
        xgp = ctx.enter_context(tc.tile_pool(name="xg", bufs=2))
        xtep = ctx.enter_context(tc.tile_pool(name="xte", bufs=2))
        hpool = ctx.enter_context(tc.tile_pool(name="hT", bufs=1))
        w1p = ctx.enter_context(tc.tile_pool(name="w1", bufs=2))
        w2p = ctx.enter_context(tc.tile_pool(name="w2", bufs=2))
        ysbp = ctx.enter_context(tc.tile_pool(name="ysb", bufs=2))
        ps1 = ctx.enter_context(tc.tile_pool(name="ps1", bufs=2, space="PSUM"))
        ps2 = ctx.enter_context(tc.tile_pool(name="ps2", bufs=4, space="PSUM"))
        psT = ctx.enter_context(tc.tile_pool(name="psT", bufs=2, space="PSUM"))

        b1sb = const.tile([128, E, NHT], F32)
        nc.sync.dma_start(out=b1sb[:], in_=b1h[:, :, :])
        b2sb = const.tile([1, E * D], FR)
        nc.sync.dma_start(out=b2sb[:], in_=b2f[:, :])
        ones = const.tile([1, 128], FR)
        nc.sync.dma_start(out=ones[:], in_=onesd[:, :])
        ident = const.tile([128, 128], FR)
        nc.sync.dma_start(out=ident[:], in_=identd[:, :])
        idxsb = const.tile([128, E, NT], mybir.dt.int32)
        nc.sync.dma_start(out=idxsb[:], in_=idxh[:, :, :])
        sidxsb = const.tile([128, E, NT], mybir.dt.int32)
        nc.sync.dma_start(out=sidxsb[:], in_=sidxh[:, :, :])
        g2sb = const.tile([128, E, NT], F32)
        nc.sync.dma_start(out=g2sb[:], in_=g2h[:, :, :])

        for e in range(E):
            # gather this expert's tokens and transpose to [d, tok]
            xte = xtep.tile([128, DS, CAP], FR)
            for tt in range(NT):
                xg = xgp.tile([128, D], FR)
                nc.gpsimd.indirect_dma_start(
                    out=xg[:], out_offset=None, in_=xrowd[:, :],
                    in_offset=bass.IndirectOffsetOnAxis(
                        ap=idxsb[:, e, tt: tt + 1], axis=0),
                )
                for ds in range(DS):
                    pt = psT.tile([128, 128], FR)
                    nc.tensor.transpose(
                        pt[:], xg[:, ds * 128: (ds + 1) * 128], ident[:])
                    nc.vector.tensor_copy(
                        xte[:, ds, tt * 128: (tt + 1) * 128], pt[:])
            # fc1
            hTe = hpool.tile([128, NHT, CAP], FR)
            for wg in range(W1G):
                w1t = w1p.tile([128, DS, 512], FR)
                nc.sync.dma_start(out=w1t[:], in_=w1h[e, wg, :, :, :])
                for hti in range(4):
                    ht = wg * 4 + hti
                    p1 = ps1.tile([128, CAP], F32)
                    for ds in range(DS):
                        nc.tensor.matmul(
                            p1[:],
                            lhsT=w1t[:, ds, hti * 128: (hti + 1) * 128],
                            rhs=xte[:, ds, :],
                            start=(ds == 0),
                            stop=(ds == DS - 1),
                        )
                    nc.scalar.activation(
                        hTe[:, ht, :], p1[:], AF.Gelu_apprx_tanh,
                        bias=b1sb[:, e, ht: ht + 1],
                    )
            # fc2 (+b2 via K=1 ones matmul) + gate scale
            ysb = ysbp.tile([128, NT, D], F32)
            for dc in range(DC):
                pst = [ps2.tile([128, 512], F32, name=f"pst{_t}", tag="pst")
                       for _t in range(NT)]
                for hq in range(HQ):
                    w2t = w2p.tile([128, 8, 512], FR)
                    nc.sync.dma_start(out=w2t[:], in_=w2h[e, dc, hq, :, :, :])
                    for hh in range(8):
                        ht = hq * 8 + hh
                        for tt in range(NT):
                            nc.tensor.matmul(
                                pst[tt][:],
                                lhsT=hTe[:, ht, tt * 128: (tt + 1) * 128],
                                rhs=w2t[:, hh, :],
                                start=(hq == 0 and hh == 0),
                                stop=False,
                            )
                for tt in range(NT):
                    nc.tensor.matmul(
                        pst[tt][:], lhsT=ones[:, :],
                        rhs=b2sb[:, e * D + dc * 512: e * D + (dc + 1) * 512],
                        start=False, stop=True,
                    )
                    nc.vector.tensor_scalar_mul(
                        ysb[:, tt, dc * 512: (dc + 1) * 512],
                        pst[tt][:], g2sb[:, e, tt: tt + 1])
            # scatter-ADD rows straight into the padded output
            for tt in range(NT):
                nc.gpsimd.indirect_dma_start(
                    out=outd[:, :],
                    out_offset=bass.IndirectOffsetOnAxis(
                        ap=sidxsb[:, e, tt: tt + 1], axis=0),
                    in_=ysb[:, tt, :], in_offset=None,
                    compute_op=OP.add,
                )
    nc.compile()
    return nc


_CACHE = {}


def _get_nc():
    if "nc" not in _CACHE:
        _CACHE["nc"] = build_nc()
    return _CACHE["nc"]


def _get_nc_routed():
    if "ncr" not in _CACHE:
        _CACHE["ncr"] = build_nc_routed()
    return _CACHE["ncr"]


def host_router(x, scale_embeddings, router_w, router_b, scale_idx):
    """Exact-fp32 router matching the reference's op order.

    Returns (gates [T, E] fp32, top2 idx [T, 2], top2 weights [T, 2]).
    """
    f = np.float32
    T = x.shape[0] * x.shape[1]
    xs = (x.astype(f, copy=False)
          + scale_embeddings[int(scale_idx)].astype(f, copy=False)[None, None, :])
    logits = (xs.reshape(T, D) @ router_w.astype(f, copy=False).T
              + router_b.astype(f, copy=False))                    # [T, E]
    # top-2 with jax.lax.top_k tie semantics (lowest index wins)
    neg = -logits
    idx = np.argsort(neg, axis=1, kind="stable")[:, :2]            # [T, 2]
    v = np.take_along_axis(logits, idx, axis=1)
    w = np.exp(v - v[:, :1])
    w = w / w.sum(axis=1, keepdims=True)
    w = w.astype(f)
    gates = np.zeros((T, E), f)
    np.put_along_axis(gates, idx, w, axis=1)
    return gates, idx, w


def _prep_shared(fc1_w, fc1_b, fc2_w, fc2_b):
    f = np.float32
    w1t = np.ascontiguousarray(fc1_w.transpose(0, 2, 1)).astype(f, copy=False)
    w1h = np.ascontiguousarray(
        w1t.reshape(E, DS, 128, W1G, 512).transpose(0, 3, 2, 1, 4))
    w2t = np.ascontiguousarray(fc2_w.transpose(0, 2, 1)).astype(f, copy=False)
    w2h = np.ascontiguousarray(
        w2t.reshape(E, HQ, 8, 128, DC, 512).transpose(0, 4, 1, 3, 2, 5))
    b1h = np.ascontiguousarray(
        fc1_b.astype(f, copy=False).reshape(E, NHT, 128).transpose(2, 0, 1))
    b2h = np.ascontiguousarray(fc2_b.astype(f, copy=False))
    return w1h, w2h, b1h, b2h


def make_in_maps(x, scale_embeddings, router_w, router_b,
                 fc1_w, fc1_b, fc2_w, fc2_b, scale_idx):
    x = np.asarray(x, np.float32)
    B, S, _ = x.shape
    T = B * S
    assert T == NCORES * TLOC and x.shape[2] == D
    w1h, w2h, b1h, b2h = _prep_shared(
        np.asarray(fc1_w), np.asarray(fc1_b),
        np.asarray(fc2_w), np.asarray(fc2_b))
    gates, _, _ = host_router(x, np.asarray(scale_embeddings),
                              np.asarray(router_w), np.asarray(router_b),
                              np.asarray(scale_idx))
    xf = x.reshape(T, D)
    in_maps = []
    for i in range(NCORES):
        xloc = xf[i * TLOC:(i + 1) * TLOC]                       # [TLOC, D]
        xT = np.ascontiguousarray(xloc.T)                        # [D, TLOC]
        xhh = np.ascontiguousarray(
            xT.reshape(DS, 128, TLOC).transpose(1, 0, 2))        # [128, DS, TLOC]
        gloc = gates[i * TLOC:(i + 1) * TLOC]                    # [TLOC, E]
        ghh = np.ascontiguousarray(
            gloc.reshape(TLOC // 128, 128, E).transpose(1, 0, 2))
        ght = np.ascontiguousarray(gloc.T)                       # [E, TLOC]
        in_maps.append({
            "xh": xhh, "w1h": w1h, "w2h": w2h, "b1h": b1h,
            "b2h": b2h, "gh": ghh, "ght": ght,
        })
    return in_maps, (B, S)


def make_in_maps_routed(x, scale_embeddings, router_w, router_b,
                        fc1_w, fc1_b, fc2_w, fc2_b, scale_idx):
    """Returns (in_maps, (B, S)) or None if any expert overflows CAP."""
    x = np.asarray(x, np.float32)
    B, S, _ = x.shape
    T = B * S
    assert T == NCORES * TLOC and x.shape[2] == D
    w1h, w2h, b1h, b2h = _prep_shared(
        np.asarray(fc1_w), np.asarray(fc1_b),
        np.asarray(fc2_w), np.asarray(fc2_b))
    gates, top_idx, top_w = host_router(
        x, np.asarray(scale_embeddings), np.asarray(router_w),
        np.asarray(router_b), np.asarray(scale_idx))
    ident = np.eye(128, dtype=np.float32)
    xf = np.ascontiguousarray(x.reshape(T, D))
    in_maps = []
    for i in range(NCORES):
        sl = slice(i * TLOC, (i + 1) * TLOC)
        xloc = np.ascontiguousarray(xf[sl])                      # [TLOC, D]
        ti, tw = top_idx[sl], top_w[sl]                          # [TLOC, 2]
        idxh = np.zeros((E, CAP), np.int32)
        sidxh = np.full((E, CAP), TLOC, np.int32)                # pad -> dummy
        g2h = np.zeros((E, CAP), np.float32)
        counts = np.zeros(E, np.int64)
        for slot in range(2):
            for t in range(TLOC):
                e = ti[t, slot]
                c = counts[e]
                if c >= CAP:
                    return None
                idxh[e, c] = t
                sidxh[e, c] = t                     # scatter-add: plain row
                g2h[e, c] = tw[t, slot]
                counts[e] = c + 1
        # device layout [128, E, NT]: list position j = tt*128 + p
        def lay(a, dt):
            return np.ascontiguousarray(
                a.reshape(E, NT, 128).transpose(2, 0, 1).astype(dt))
        in_maps.append({
            "xrowd": xloc, "w1h": w1h, "w2h": w2h, "b1h": b1h,
            "b2f": b2h.reshape(1, E * D), "idxh": lay(idxh, np.int32),
            "sidxh": lay(sidxh, np.int32), "g2h": lay(g2h, np.float32),
            "identd": ident, "onesd": np.ones((1, 128), np.float32),
        })
    return in_maps, (B, S)


def kernel(x, scale_embeddings, router_w, router_b,
           fc1_w, fc1_b, fc2_w, fc2_b, scale_idx):
    args = (x, scale_embeddings, router_w, router_b,
            fc1_w, fc1_b, fc2_w, fc2_b, scale_idx)
    routed = make_in_maps_routed(*args)
    if routed is not None:
        in_maps, (B, S) = routed
        nc = _get_nc_routed()
        res = run_bass_kernel_spmd(nc, in_maps, core_ids=list(range(NCORES)))
        parts = [res.results[i]["outd"][:TLOC] for i in range(NCORES)]
        return np.concatenate(parts, 0).reshape(B, S, D)
    # capacity overflow (practically impossible): dense fallback
    in_maps, (B, S) = make_in_maps(*args)
    nc = _get_nc()
    res = run_bass_kernel_spmd(nc, in_maps, core_ids=list(range(NCORES)))
    parts = []
    for i in range(NCORES):
        o = res.results[i]["outd"]                               # [NCH,128,TT,DC,512]
        parts.append(o.transpose(0, 2, 1, 3, 4).reshape(TLOC, D))
    return np.concatenate(parts, 0).reshape(B, S, D)


# revision 31
# speedup vs baseline: 1.1524x; 1.1524x over previous
"""MoE FFN (8 experts, top-2) Trainium2 Bass kernel.

Strategy: token-sharded data parallel over 8 cores. Core i handles tokens
[i*1024, (i+1)*1024) of the flattened [B*S=8192, D=1024] input. The tiny
router (0.06% of FLOPs) runs on host in exact fp32 (matching the reference's
op order, so top-2 selection is bit-stable); the dense FFN compute runs on
device: for each of the 8 experts, hT = gelu_tanh(w1 @ x + b1) over all local
tokens, then out += gate_e * (w2.T @ hT + b2), accumulated in SBUF.
All FFN matmuls use float32r (fp32 bits, 4x-faster PE mode, full rate at
moving dim >= 256).

Everything lives in [feature, token] layout on chip so no transposes are
needed: fc1 consumes xT tiles as the moving operand and w1T tiles as
stationary; fc2 consumes hT tiles as stationary and w2T tiles as moving,
producing [token, d] PSUM tiles so the per-token gate is a per-partition
scalar multiply fused with the cross-expert accumulation (one DVE op).
"""

import numpy as np
from contextlib import ExitStack

import concourse.bass as bass
import concourse.bacc as bacc
import concourse.tile as tile
from concourse import mybir
from concourse.bass_utils import run_bass_kernel_spmd

FR = mybir.dt.float32r
F32 = mybir.dt.float32
AF = mybir.ActivationFunctionType
OP = mybir.AluOpType

NCORES = 8
E = 8            # experts
D = 1024         # model dim
H = 4096         # hidden dim
TLOC = 1024      # tokens per core
CHUNK = 512      # tokens per hT block
NCH = TLOC // CHUNK
TT = CHUNK // 128        # token tiles per chunk (4)
DS = D // 128            # d sub-blocks (8)
NHT = H // 128           # h tiles (32)
W1G = H // 512           # 8 w1 DMA groups per expert, each [128, DS, 512]
DC = D // 512            # 2 output d chunks
HQ = 4                   # w2 h-quarters, each 8 h-tiles


def build_nc():
    nc = bacc.Bacc("TRN2", target_bir_lowering=False, debug=False,
                   num_devices=NCORES)
    xh = nc.dram_tensor("xh", [128, DS, TLOC], FR, kind="ExternalInput")
    w1h = nc.dram_tensor("w1h", [E, W1G, 128, DS, 512], FR, kind="ExternalInput")
    w2h = nc.dram_tensor("w2h", [E, DC, HQ, 128, 8, 512], FR, kind="ExternalInput")
    b1h = nc.dram_tensor("b1h", [128, E, NHT], F32, kind="ExternalInput")
    b2h = nc.dram_tensor("b2h", [E, D], FR, kind="ExternalInput")
    # host-computed gates: gh[p, tt_global, e] (token t = tt_global*128 + p)
    gh = nc.dram_tensor("gh", [128, TLOC // 128, E], F32, kind="ExternalInput")
    # transposed gates for the fc2-bias rank-1 term: ght[e, tok]
    ght = nc.dram_tensor("ght", [E, TLOC], FR, kind="ExternalInput")
    outd = nc.dram_tensor("outd", [NCH, 128, TT, DC, 512], F32,
                          kind="ExternalOutput")

    with tile.TileContext(nc) as tc, ExitStack() as ctx:
        const = ctx.enter_context(tc.tile_pool(name="const", bufs=1))
        hpool = ctx.enter_context(tc.tile_pool(name="hT", bufs=1))
        apool = ctx.enter_context(tc.tile_pool(name="oacc", bufs=2))
        w1p = ctx.enter_context(tc.tile_pool(name="w1", bufs=2))
        w2p = ctx.enter_context(tc.tile_pool(name="w2", bufs=2))
        ps1 = ctx.enter_context(tc.tile_pool(name="ps1", bufs=2, space="PSUM"))
        ps2 = ctx.enter_context(tc.tile_pool(name="ps2", bufs=6, space="PSUM"))

        # --- resident tensors ---
        xsb = const.tile([128, DS, TLOC], FR)
        nc.sync.dma_start(out=xsb[:], in_=xh[:, :, :])
        b1sb = const.tile([128, E, NHT], F32)
        nc.sync.dma_start(out=b1sb[:], in_=b1h[:, :, :])
        b2sb = const.tile([E, D], FR)
        nc.sync.dma_start(out=b2sb[:], in_=b2h[:, :])
        gsb = const.tile([128, TLOC // 128, E], F32)
        nc.sync.dma_start(out=gsb[:], in_=gh[:, :, :])
        gtsb = const.tile([E, TLOC], FR)
        nc.sync.dma_start(out=gtsb[:], in_=ght[:, :])

        for c in range(NCH):
            t0 = c * CHUNK
            # init oacc with the fc2 bias term: oacc[t, d] = sum_e g_e(t) b2_e(d)
            oacc = apool.tile([128, TT, DC, 512], F32)
            for tt in range(TT):
                for dc in range(DC):
                    pb = ps2.tile([128, 512], F32, name=f"pb{tt}_{dc}", tag="pst")
                    nc.tensor.matmul(
                        pb[:],
                        lhsT=gtsb[:, t0 + tt * 128: t0 + (tt + 1) * 128],
                        rhs=b2sb[:, dc * 512: (dc + 1) * 512],
                        start=True, stop=True,
                    )
                    nc.vector.tensor_copy(oacc[:, tt, dc, :], pb[:])

            for e in range(E):
                # ---------------- fc1: hT[h, tok] = gelu(w1 @ x + b1) --------
                hT = hpool.tile([128, NHT, CHUNK], FR)
                for wg in range(W1G):  # 8 groups x 4 h-tiles
                    w1t = w1p.tile([128, DS, 512], FR)
                    nc.sync.dma_start(out=w1t[:], in_=w1h[e, wg, :, :, :])
                    for hti in range(4):
                        ht = wg * 4 + hti
                        p1 = ps1.tile([128, 512], F32)
                        for ds in range(DS):
                            nc.tensor.matmul(
                                p1[:, :CHUNK],
                                lhsT=w1t[:, ds, hti * 128: (hti + 1) * 128],
                                rhs=xsb[:, ds, t0: t0 + CHUNK],
                                start=(ds == 0),
                                stop=(ds == DS - 1),
                            )
                        nc.scalar.activation(
                            hT[:, ht, :], p1[:, :CHUNK], AF.Gelu_apprx_tanh,
                            bias=b1sb[:, e, ht: ht + 1],
                        )
                # ---------------- fc2: out[tok, d] += g_e * (hT.T @ w2) ------
                for dc in range(DC):
                    pst = [ps2.tile([128, 512], F32, name=f"pst{_t}", tag="pst")
                           for _t in range(TT)]
                    for hq in range(HQ):
                        w2t = w2p.tile([128, 8, 512], FR)
                        nc.sync.dma_start(out=w2t[:], in_=w2h[e, dc, hq, :, :, :])
                        for hh in range(8):
                            ht = hq * 8 + hh
                            for tt in range(TT):
                                nc.tensor.matmul(
                                    pst[tt][:],
                                    lhsT=hT[:, ht, tt * 128: (tt + 1) * 128],
                                    rhs=w2t[:, hh, :],
                                    start=(hq == 0 and hh == 0),
                                    stop=(hq == HQ - 1 and hh == 7),
                                )
                    for tt in range(TT):
                        nc.vector.scalar_tensor_tensor(
                            out=oacc[:, tt, dc, :],
                            in0=pst[tt][:],
                            scalar=gsb[:, (t0 // 128) + tt, e: e + 1],
                            in1=oacc[:, tt, dc, :],
                            op0=OP.mult,
                            op1=OP.add,
                        )
            nc.sync.dma_start(out=outd[c, :, :, :, :], in_=oacc[:])
    nc.compile()
    return nc


CAP = 384                # routed capacity per (core, expert): 3 token tiles
NT = CAP // 128
TLOC1 = TLOC + 1         # +1 dummy row for padded scatter slots


def build_nc_routed():
    """Routed variant: each expert computes only its own tokens.

    Host supplies per-expert gather indices (into the core's local x rows),
    scatter indices (row in the padded output; CAP-padding slots point at the
    dummy row TLOC), and gathered gates. Device: indirect-DMA gather -> PE
    transpose -> fc1 -> fc2 (+bias via K=1 ones matmul) -> gate-scale ->
    indirect scatter-ADD straight into the (pre-zeroed) padded output.
    """
    nc = bacc.Bacc("TRN2", target_bir_lowering=False, debug=False,
                   num_devices=NCORES)
    xrowd = nc.dram_tensor("xrowd", [TLOC, D], FR, kind="ExternalInput")
    w1h = nc.dram_tensor("w1h", [E, W1G, 128, DS, 512], FR, kind="ExternalInput")
    w2h = nc.dram_tensor("w2h", [E, DC, HQ, 128, 8, 512], FR, kind="ExternalInput")
    b1h = nc.dram_tensor("b1h", [128, E, NHT], F32, kind="ExternalInput")
    b2f = nc.dram_tensor("b2f", [1, E * D], FR, kind="ExternalInput")
    onesd = nc.dram_tensor("onesd", [1, 128], FR, kind="ExternalInput")
    idxh = nc.dram_tensor("idxh", [128, E, NT], mybir.dt.int32,
                          kind="ExternalInput")
    sidxh = nc.dram_tensor("sidxh", [128, E, NT], mybir.dt.int32,
                           kind="ExternalInput")
    g2h = nc.dram_tensor("g2h", [128, E, NT], F32, kind="ExternalInput")
    identd = nc.dram_tensor("identd", [128, 128], FR, kind="ExternalInput")
    outd = nc.dram_tensor("outd", [TLOC, D], F32, kind="ExternalOutput")
    out2d = nc.dram_tensor("out2d", [2 * TLOC1, D], F32, kind="Internal")

    with tile.TileContext(nc) as tc, ExitStack() as ctx:
        const = ctx.enter_context(tc.tile_pool(name="const", bufs=1))
        xgp = ctx.enter_context(tc.tile_pool(name="xg", bufs=2))
        xtep = ctx.enter_context(tc.tile_pool(name="xte", bufs=2))
        hpool = ctx.enter_context(tc.tile_pool(name="hT", bufs=1))
        w1p = ctx.enter_context(tc.tile_pool(name="w1", bufs=2))
        w2p = ctx.enter_context(tc.tile_pool(name="w2", bufs=2))
        ysbp = ctx.enter_context(tc.tile_pool(name="ysb", bufs=2))
        cmb = ctx.enter_context(tc.tile_pool(name="cmb", bufs=2))
        b2p = ctx.enter_context(tc.tile_pool(name="b2p", bufs=2))
        ps1 = ctx.enter_context(tc.tile_pool(name="ps1", bufs=2, space="PSUM"))
        ps2 = ctx.enter_context(tc.tile_pool(name="ps2", bufs=4, space="PSUM"))
        psT = ctx.enter_context(tc.tile_pool(name="psT", bufs=2, space="PSUM"))

        b1sb = const.tile([128, E, NHT], F32)
        nc.sync.dma_start(out=b1sb[:], in_=b1h[:, :, :])
        ones = const.tile([1, 128], FR)
        nc.sync.dma_start(out=ones[:], in_=onesd[:, :])
        ident = const.tile([128, 128], FR)
        nc.sync.dma_start(out=ident[:], in_=identd[:, :])
        idxsb = const.tile([128, E, NT], mybir.dt.int32)
        nc.sync.dma_start(out=idxsb[:], in_=idxh[:, :, :])
        sidxsb = const.tile([128, E, NT], mybir.dt.int32)
        nc.sync.dma_start(out=sidxsb[:], in_=sidxh[:, :, :])
        g2sb = const.tile([128, E, NT], F32)
        nc.sync.dma_start(out=g2sb[:], in_=g2h[:, :, :])

        for e in range(E):
            b2sb = b2p.tile([1, D], FR)
            nc.sync.dma_start(out=b2sb[:], in_=b2f[0:1, e * D:(e + 1) * D])
            # gather this expert's tokens and transpose to [d, tok]
            xte = xtep.tile([128, DS, CAP], FR)
            for tt in range(NT):
                xg = xgp.tile([128, D], FR)
                nc.gpsimd.indirect_dma_start(
                    out=xg[:], out_offset=None, in_=xrowd[:, :],
                    in_offset=bass.IndirectOffsetOnAxis(
                        ap=idxsb[:, e, tt: tt + 1], axis=0),
                )
                for ds in range(DS):
                    pt = psT.tile([128, 128], FR)
                    nc.tensor.transpose(
                        pt[:], xg[:, ds * 128: (ds + 1) * 128], ident[:])
                    nc.vector.tensor_copy(
                        xte[:, ds, tt * 128: (tt + 1) * 128], pt[:])
            # fc1
            hTe = hpool.tile([128, NHT, CAP], FR)
            for wg in range(W1G):
                w1t = w1p.tile([128, DS, 512], FR)
                nc.sync.dma_start(out=w1t[:], in_=w1h[e, wg, :, :, :])
                for hti in range(4):
                    ht = wg * 4 + hti
                    p1 = ps1.tile([128, CAP], F32)
                    for ds in range(DS):
                        nc.tensor.matmul(
                            p1[:],
                            lhsT=w1t[:, ds, hti * 128: (hti + 1) * 128],
                            rhs=xte[:, ds, :],
                            start=(ds == 0),
                            stop=(ds == DS - 1),
                        )
                    nc.scalar.activation(
                        hTe[:, ht, :], p1[:], AF.Gelu_apprx_tanh,
                        bias=b1sb[:, e, ht: ht + 1],
                    )
            # fc2 (+b2 via K=1 ones matmul) + gate scale
            ysb = ysbp.tile([128, NT, D], F32)
            for dc in range(DC):
                pst = [ps2.tile([128, 512], F32, name=f"pst{_t}", tag="pst")
                       for _t in range(NT)]
                for hq in range(HQ):
                    w2t = w2p.tile([128, 8, 512], FR)
                    nc.sync.dma_start(out=w2t[:], in_=w2h[e, dc, hq, :, :, :])
                    for hh in range(8):
                        ht = hq * 8 + hh
                        for tt in range(NT):
                            nc.tensor.matmul(
                                pst[tt][:],
                                lhsT=hTe[:, ht, tt * 128: (tt + 1) * 128],
                                rhs=w2t[:, hh, :],
                                start=(hq == 0 and hh == 0),
                                stop=False,
                            )
                for tt in range(NT):
                    nc.tensor.matmul(
                        pst[tt][:], lhsT=ones[:, :],
                        rhs=b2sb[:, dc * 512: (dc + 1) * 512],
                        start=False, stop=True,
                    )
                    nc.vector.tensor_scalar_mul(
                        ysb[:, tt, dc * 512: (dc + 1) * 512],
                        pst[tt][:], g2sb[:, e, tt: tt + 1])
            # scatter rows into the slot planes
            for tt in range(NT):
                nc.gpsimd.indirect_dma_start(
                    out=out2d[:, :],
                    out_offset=bass.IndirectOffsetOnAxis(
                        ap=sidxsb[:, e, tt: tt + 1], axis=0),
                    in_=ysb[:, tt, :], in_offset=None,
                )
        # combine: out = plane0 + plane1 (bias already folded into ysb)
        for t8 in range(TLOC // 128):
            p0 = cmb.tile([128, D], F32)
            nc.sync.dma_start(out=p0[:], in_=out2d[t8 * 128:(t8 + 1) * 128, :])
            p1t = cmb.tile([128, D], F32)
            nc.sync.dma_start(
                out=p1t[:],
                in_=out2d[TLOC1 + t8 * 128: TLOC1 + (t8 + 1) * 128, :])
            outt = cmb.tile([128, D], F32)
            nc.vector.tensor_add(outt[:], p0[:], p1t[:])
            nc.sync.dma_start(out=outd[t8 * 128:(t8 + 1) * 128, :], in_=outt[:])
    nc.compile()
    return nc


_CACHE = {}


def _get_nc():
    if "nc" not in _CACHE:
        _CACHE["nc"] = build_nc()
    return _CACHE["nc"]


def _get_nc_routed():
    if "ncr" not in _CACHE:
        _CACHE["ncr"] = build_nc_routed()
    return _CACHE["ncr"]


def host_router(x, scale_embeddings, router_w, router_b, scale_idx):
    """Exact-fp32 router matching the reference's op order.

    Returns (gates [T, E] fp32, top2 idx [T, 2], top2 weights [T, 2]).
    """
    f = np.float32
    T = x.shape[0] * x.shape[1]
    xs = (x.astype(f, copy=False)
          + scale_embeddings[int(scale_idx)].astype(f, copy=False)[None, None, :])
    logits = (xs.reshape(T, D) @ router_w.astype(f, copy=False).T
              + router_b.astype(f, copy=False))                    # [T, E]
    # top-2 with jax.lax.top_k tie semantics (lowest index wins)
    neg = -logits
    idx = np.argsort(neg, axis=1, kind="stable")[:, :2]            # [T, 2]
    v = np.take_along_axis(logits, idx, axis=1)
    w = np.exp(v - v[:, :1])
    w = w / w.sum(axis=1, keepdims=True)
    w = w.astype(f)
    gates = np.zeros((T, E), f)
    np.put_along_axis(gates, idx, w, axis=1)
    return gates, idx, w


def _prep_shared(fc1_w, fc1_b, fc2_w, fc2_b):
    f = np.float32
    w1t = np.ascontiguousarray(fc1_w.transpose(0, 2, 1)).astype(f, copy=False)
    w1h = np.ascontiguousarray(
        w1t.reshape(E, DS, 128, W1G, 512).transpose(0, 3, 2, 1, 4))
    w2t = np.ascontiguousarray(fc2_w.transpose(0, 2, 1)).astype(f, copy=False)
    w2h = np.ascontiguousarray(
        w2t.reshape(E, HQ, 8, 128, DC, 512).transpose(0, 4, 1, 3, 2, 5))
    b1h = np.ascontiguousarray(
        fc1_b.astype(f, copy=False).reshape(E, NHT, 128).transpose(2, 0, 1))
    b2h = np.ascontiguousarray(fc2_b.astype(f, copy=False))
    return w1h, w2h, b1h, b2h


def make_in_maps(x, scale_embeddings, router_w, router_b,
                 fc1_w, fc1_b, fc2_w, fc2_b, scale_idx):
    x = np.asarray(x, np.float32)
    B, S, _ = x.shape
    T = B * S
    assert T == NCORES * TLOC and x.shape[2] == D
    w1h, w2h, b1h, b2h = _prep_shared(
        np.asarray(fc1_w), np.asarray(fc1_b),
        np.asarray(fc2_w), np.asarray(fc2_b))
    gates, _, _ = host_router(x, np.asarray(scale_embeddings),
                              np.asarray(router_w), np.asarray(router_b),
                              np.asarray(scale_idx))
    xf = x.reshape(T, D)
    in_maps = []
    for i in range(NCORES):
        xloc = xf[i * TLOC:(i + 1) * TLOC]                       # [TLOC, D]
        xT = np.ascontiguousarray(xloc.T)                        # [D, TLOC]
        xhh = np.ascontiguousarray(
            xT.reshape(DS, 128, TLOC).transpose(1, 0, 2))        # [128, DS, TLOC]
        gloc = gates[i * TLOC:(i + 1) * TLOC]                    # [TLOC, E]
        ghh = np.ascontiguousarray(
            gloc.reshape(TLOC // 128, 128, E).transpose(1, 0, 2))
        ght = np.ascontiguousarray(gloc.T)                       # [E, TLOC]
        in_maps.append({
            "xh": xhh, "w1h": w1h, "w2h": w2h, "b1h": b1h,
            "b2h": b2h, "gh": ghh, "ght": ght,
        })
    return in_maps, (B, S)


def make_in_maps_routed(x, scale_embeddings, router_w, router_b,
                        fc1_w, fc1_b, fc2_w, fc2_b, scale_idx):
    """Returns (in_maps, (B, S)) or None if any expert overflows CAP."""
    x = np.asarray(x, np.float32)
    B, S, _ = x.shape
    T = B * S
    assert T == NCORES * TLOC and x.shape[2] == D
    w1h, w2h, b1h, b2h = _prep_shared(
        np.asarray(fc1_w), np.asarray(fc1_b),
        np.asarray(fc2_w), np.asarray(fc2_b))
    gates, top_idx, top_w = host_router(
        x, np.asarray(scale_embeddings), np.asarray(router_w),
        np.asarray(router_b), np.asarray(scale_idx))
    ident = np.eye(128, dtype=np.float32)
    xf = np.ascontiguousarray(x.reshape(T, D))
    in_maps = []
    for i in range(NCORES):
        sl = slice(i * TLOC, (i + 1) * TLOC)
        xloc = np.ascontiguousarray(xf[sl])                      # [TLOC, D]
        ti, tw = top_idx[sl], top_w[sl]                          # [TLOC, 2]
        idxh = np.zeros((E, CAP), np.int32)
        sidxh = np.full((E, CAP), TLOC, np.int32)                # pad -> dummy
        g2h = np.zeros((E, CAP), np.float32)
        counts = np.zeros(E, np.int64)
        for slot in range(2):
            for t in range(TLOC):
                e = ti[t, slot]
                c = counts[e]
                if c >= CAP:
                    return None
                idxh[e, c] = t
                sidxh[e, c] = slot * TLOC1 + t
                g2h[e, c] = tw[t, slot]
                counts[e] = c + 1
        # device layout [128, E, NT]: list position j = tt*128 + p
        def lay(a, dt):
            return np.ascontiguousarray(
                a.reshape(E, NT, 128).transpose(2, 0, 1).astype(dt))
        in_maps.append({
            "xrowd": xloc, "w1h": w1h, "w2h": w2h, "b1h": b1h,
            "b2f": b2h.reshape(1, E * D), "idxh": lay(idxh, np.int32),
            "sidxh": lay(sidxh, np.int32), "g2h": lay(g2h, np.float32),
            "identd": ident, "onesd": np.ones((1, 128), np.float32),
        })
    return in_maps, (B, S)


def kernel(x, scale_embeddings, router_w, router_b,
           fc1_w, fc1_b, fc2_w, fc2_b, scale_idx):
    args = (x, scale_embeddings, router_w, router_b,
            fc1_w, fc1_b, fc2_w, fc2_b, scale_idx)
    routed = make_in_maps_routed(*args)
    if routed is not None:
        in_maps, (B, S) = routed
        nc = _get_nc_routed()
        res = run_bass_kernel_spmd(nc, in_maps, core_ids=list(range(NCORES)))
        parts = [res.results[i]["outd"] for i in range(NCORES)]
        return np.concatenate(parts, 0).reshape(B, S, D)
    # capacity overflow (practically impossible): dense fallback
    in_maps, (B, S) = make_in_maps(*args)
    nc = _get_nc()
    res = run_bass_kernel_spmd(nc, in_maps, core_ids=list(range(NCORES)))
    parts = []
    for i in range(NCORES):
        o = res.results[i]["outd"]                               # [NCH,128,TT,DC,512]
        parts.append(o.transpose(0, 2, 1, 3, 4).reshape(TLOC, D))
    return np.concatenate(parts, 0).reshape(B, S, D)
